# revision 9
# baseline (speedup 1.0000x reference)
"""GNN message-passing kernel for Trainium2 (8 NeuronCores).

out[v] = tanh( sum_w W[w] @ sum_{edges e: v_e=v, widx_e=w} x[u_e] )

Strategy (dest-sharded, gather-only, matmul segment-sum):
  - Nodes (destinations) sharded across 8 cores: core c owns v in
    [c*12500, (c+1)*12500).  x cast to fp16 host-side.
  - Edges bucketed host-side by (core, w, half, u_window, quad) where
    quad = 512 consecutive local dests; each cell padded to a multiple
    of 128 slots (cap shared across cores = SPMD).
  - Device, per (w, half): 4 dma_gather ops (one per 25k-row u-window)
    pull x[u_e] rows into SBUF staging in slot order.  Per quad q:
    chunks of 128 gathered rows are segment-summed via the Tensor
    engine:  S^T[j, seg] += G_chunk^T(stationary) @ onehot(segid)
    where onehot is built on DVE by comparing an iota row against the
    per-slot segment id (pad slots get segid=512 -> zero column).
    S^T accumulates in a PSUM bank [128, 512] across the quad's
    chunks; ACT copies it to SBUF (fp16); 4 weight matmuls
    (lhsT = S^T slice [j, v], rhs = W^T[j, i]) produce out[v, i]
    partials, accumulated over w into an SBUF f32 accumulator.
  - Final: tanh on ACT, DMA out.  No scatter, no S tables in HBM.
"""
import os
import numpy as np
import ml_dtypes

import concourse.bass as bass
import concourse.bacc as bacc
import concourse.mybir as mybir
import concourse.tile as tile
from concourse.bass_utils import run_bass_kernel_spmd

# problem shape (hardcoded per contract)
N, D, E, NW = 100000, 128, 2000000, 8
C = 8                  # cores
NPC = N // C           # 12500 dest nodes per core
WIN = 25000            # u gather window (int16-addressable rows)
NWIN = 4               # windows covering N
QS = 512               # dest quad size (one PSUM bank of segments)
NQ = 25                # quads per core (25*512 = 12800 >= 12500)
HALVES = [range(0, 13), range(13, 25)]  # quad halves per w
PAD_SEG = 512.0        # segid for pad slots (outside iota 0..511)

LAST_RESULTS = None    # BassKernelResults of the most recent run


def _wrap16(flat):
    """[n] -> [128, n/16] idx layout: position i at [i%16, i//16], replicated 8x."""
    base = flat.reshape(-1, 16).T  # [16, n/16]
    return np.tile(base, (8, 1))


def _prep(u, v, widx):
    """Bucket edges per core by (w, half, uwin, quad); shared caps across cores.

    Returns (caps[NW,NWIN,NQ] in chunks, gidx_all, segid_all, TOT, NCOL).
    """
    core = v // NPC
    vloc = v - core * NPC
    q = vloc // QS
    uw = u // WIN
    w = widx

    # counts per (core, w, uw, q) -> shared caps (in 128-chunks)
    lin = ((core * NW + w) * NWIN + uw) * NQ + q
    cnt = np.bincount(lin, minlength=C * NW * NWIN * NQ).reshape(C, NW, NWIN, NQ)
    chunks = -(-cnt.max(axis=0) // 128)          # [NW, NWIN, NQ] ceil
    chunks = np.maximum(chunks, 0)
    caps = chunks * 128

    # slot order: (w, half, uw, q-within-half); seg-col order: (w, q, uw)
    slot_cells = []   # (w, uw, q) in slot order
    for ww in range(NW):
        for half in HALVES:
            for uwin in range(NWIN):
                for qq in half:
                    slot_cells.append((ww, uwin, qq))
    col_cells = []
    for ww in range(NW):
        for qq in range(NQ):
            for uwin in range(NWIN):
                col_cells.append((ww, uwin, qq))

    slot_base = {}
    off = 0
    for cell in slot_cells:
        slot_base[cell] = off
        off += int(caps[cell[0], cell[1], cell[2]])
    TOT = off
    col_base = {}
    coff = 0
    for cell in col_cells:
        col_base[cell] = coff
        coff += int(chunks[cell[0], cell[1], cell[2]])
    NCOL = coff

    # per-chunk span union across cores -> (base, width) per segid column
    # col order: (w, q, uw, k);  smin/smax per (core, col)
    ncol_arr = np.zeros((C, coff, 2), np.int32)
    ncol_arr[:, :, 0] = QS    # smin init
    ncol_arr[:, :, 1] = -1    # smax init
    gidx_all, segid_all = [], []
    for cc in range(C):
        sel = core == cc
        ue = u[sel]
        vl = vloc[sel]
        we = w[sel]
        uwe = uw[sel]
        qe = q[sel]
        key = ((we * NWIN + uwe) * NQ + qe)
        order = np.lexsort((vl, key))
        ue, vl, we, uwe, qe, key = (a[order] for a in (ue, vl, we, uwe, qe, key))
        g_flat = np.zeros(TOT, np.int16)
        s_flat = np.full(NCOL * 128, PAD_SEG, np.float32)
        # per-cell fill via searchsorted on sorted keys
        bounds = np.searchsorted(key, np.arange(NW * NWIN * NQ + 1))
        for ww in range(NW):
            for uwin in range(NWIN):
                for qq in range(NQ):
                    b = (ww * NWIN + uwin) * NQ + qq
                    lo, hi = bounds[b], bounds[b + 1]
                    n = hi - lo
                    if n == 0:
                        continue
                    sb = slot_base[(ww, uwin, qq)]
                    cb = col_base[(ww, uwin, qq)]
                    assert n <= 128 * chunks[ww, uwin, qq]
                    g_flat[sb:sb + n] = (ue[lo:hi] - uwin * WIN).astype(np.int16)
                    s_flat[cb * 128:cb * 128 + n] = (vl[lo:hi] - qq * QS).astype(np.float32)
        gidx_all.append(_wrap16(g_flat))
        segid_all.append(s_flat.reshape(NCOL, 128))
        valid = s_flat.reshape(NCOL, 128) < QS
        sm = np.where(valid, s_flat.reshape(NCOL, 128), QS).min(axis=1)
        sx = np.where(valid, s_flat.reshape(NCOL, 128), -1).max(axis=1)
        ncol_arr[cc, :, 0] = sm
        ncol_arr[cc, :, 1] = sx
    smin = ncol_arr[:, :, 0].min(axis=0)
    smax = ncol_arr[:, :, 1].max(axis=0)
    smax = np.maximum(smax, 0)
    smin = np.minimum(smin, smax)
    base = np.minimum(smin // 128 * 128, QS - 128)
    width = (-(-(smax + 1 - base) // 128) * 128).clip(128, QS)
    base = np.minimum(base, QS - width)   # keep base+width <= QS
    # first chunk of every cell stays full-width (initializes its PSUM bank)
    first_cols = np.array(sorted(col_base.values()), int)
    first_cols = first_cols[first_cols < coff]
    base[first_cols] = 0
    width[first_cols] = QS
    # rebase segid planes; pad slots -> QS (never matches iota 0..width-1)
    segid_fin = []
    for cc in range(C):
        sp = segid_all[cc]            # [NCOL, 128]
        reb = np.where(sp < QS, sp - base[:, None], QS).astype(np.float32)
        segid_fin.append(np.ascontiguousarray(reb.T))
    segid_all = segid_fin
    return (chunks, slot_base, col_base, TOT, NCOL, gidx_all, segid_all,
            base.astype(int), width.astype(int))


def _build_nc(chunks, slot_base, col_base, TOT, NCOL, cbase, cwidth):
    nc = bacc.Bacc("TRN2", target_bir_lowering=False, debug=False, num_devices=C,
                   num_swdge_queues=4)
    fp16 = mybir.dt.float16
    f32 = mybir.dt.float32
    fp8 = mybir.dt.float8e4
    x_d = nc.dram_tensor("x16", [N, D], fp16, kind="ExternalInput")
    wt_d = nc.dram_tensor("wt", [D, NW, D], fp16, kind="ExternalInput")
    gidx_d = nc.dram_tensor("gidx", [128, TOT // 16], mybir.dt.int16,
                            kind="ExternalInput")
    segid_d = nc.dram_tensor("segid", [128, NCOL], f32, kind="ExternalInput")
    out_d = nc.dram_tensor("out", [NQ * QS, D], f32, kind="ExternalOutput")

    # per-op (w, half, uw): slot count and per-cell staging column offsets
    op_slots = {}
    stg_col = {}
    for ww in range(NW):
        for hh, half in enumerate(HALVES):
            for uwin in range(NWIN):
                s = 0
                for qq in half:
                    stg_col[(ww, uwin, qq)] = s
                    s += int(chunks[ww, uwin, qq])
                op_slots[(ww, hh, uwin)] = s * 128
    CMAX = max(op_slots.values()) // 128
    MAXCH = 0
    for ww in range(NW):
        for qq in range(NQ):
            MAXCH = max(MAXCH, int(chunks[ww, :, qq].sum()))

    with tile.TileContext(nc) as tc:
        with (
            tc.tile_pool(name="const", bufs=1) as constp,
            tc.tile_pool(name="outsb", bufs=1) as outp,
            tc.tile_pool(name="gip", bufs=8) as gip,
            tc.tile_pool(name="stgp", bufs=8) as stgp,
            tc.tile_pool(name="indp", bufs=12) as indp,
            tc.tile_pool(name="stp", bufs=4) as stp,
            tc.tile_pool(name="psA", bufs=3, space="PSUM") as psA,
            tc.tile_pool(name="psB", bufs=3, space="PSUM") as psB,
        ):
            iota_t = constp.tile([128, QS], fp16)
            nc.gpsimd.iota(iota_t[:], pattern=[[1, QS]], base=0,
                           channel_multiplier=0,
                           allow_small_or_imprecise_dtypes=True)
            wt_t = constp.tile([128, NW, D], fp16)
            nc.sync.dma_start(out=wt_t[:], in_=wt_d[:])
            segid_t = constp.tile([128, NCOL], f32)
            nc.sync.dma_start(out=segid_t[:], in_=segid_d[:])
            out_sb = []
            for qq in range(NQ):
                t = outp.tile([128, QS], f32, tag=f"out{qq}")
                nc.vector.memset(t[:], 0.0)
                out_sb.append(t)

            gcol = [0]  # running segid column, (w, q, uw, k) order

            slot_off = 0
            for ww in range(NW):
                for hh, half in enumerate(HALVES):
                    stg = {}
                    for uwin in range(NWIN):
                        nslots = op_slots[(ww, hh, uwin)]
                        if nslots == 0:
                            continue
                        gi = gip.tile([128, CMAX * 8], mybir.dt.int16, tag="gi")
                        nc.sync.dma_start(
                            out=gi[:, :nslots // 16],
                            in_=gidx_d[:, slot_off // 16:(slot_off + nslots) // 16])
                        st = stgp.tile([128, CMAX, D], fp16, tag="stg")
                        nc.gpsimd.dma_gather(
                            st[:, :nslots // 128, :],
                            x_d[uwin * WIN:min((uwin + 1) * WIN, N)],
                            gi[:, :nslots // 16], nslots, nslots, D,
                            single_packet=False, queue_num=uwin)
                        stg[uwin] = st
                        slot_off += nslots
                    for qq in half:
                        cell_list = []
                        for uwin in range(NWIN):
                            nch = int(chunks[ww, uwin, qq])
                            for k in range(nch):
                                cell_list.append((uwin, stg_col[(ww, uwin, qq)] + k))
                        if not cell_list:
                            continue
                        acc = psA.tile([128, QS], f32, tag="acc")
                        for i, (uwin, colk) in enumerate(cell_list):
                            g = gcol[0]
                            gcol[0] += 1
                            b, wdt = int(cbase[g]), int(cwidth[g])
                            if i == 0:
                                assert (b, wdt) == (0, QS)
                            ind = indp.tile([128, QS], fp16, tag="ind")
                            eng = nc.gpsimd if (g % 6 == 5) else nc.vector
                            eng.tensor_scalar(
                                ind[:, :wdt], iota_t[:, :wdt],
                                segid_t[:, g:g + 1], None,
                                mybir.AluOpType.is_equal)
                            nc.tensor.matmul(
                                out=acc[:, b:b + wdt],
                                lhsT=stg[uwin][:, colk, :],
                                rhs=ind[:, :wdt],
                                start=(i == 0), stop=(i == len(cell_list) - 1),
                                skip_group_check=True)
                        stq = stp.tile([128, QS], fp16, tag="st")
                        nc.scalar.activation(stq[:], acc[:],
                                             mybir.ActivationFunctionType.Copy)
                        wp = psB.tile([128, QS], f32, tag="wp")
                        for vt in range(4):
                            nc.tensor.matmul(
                                out=wp[:, vt * 128:(vt + 1) * 128],
                                lhsT=stq[:, vt * 128:(vt + 1) * 128],
                                rhs=wt_t[:, ww, :], start=True, stop=True)
                        nc.vector.tensor_tensor(
                            out=out_sb[qq][:], in0=out_sb[qq][:], in1=wp[:],
                            op=mybir.AluOpType.add)

            # final tanh + store
            with tc.tile_pool(name="fin", bufs=4) as finp:
                for qq in range(NQ):
                    for vt in range(4):
                        ot = finp.tile([128, 128], f32, tag="ot")
                        nc.scalar.activation(
                            ot[:], out_sb[qq][:, vt * 128:(vt + 1) * 128],
                            mybir.ActivationFunctionType.Tanh)
                        r = qq * QS + vt * 128
                        nc.sync.dma_start(out=out_d[r:r + 128, :], in_=ot[:])

    nc.compile()
    return nc


def kernel(x, W, u, v, widx):
    global LAST_RESULTS
    x = np.asarray(x, dtype=np.float32)
    W = np.asarray(W, dtype=np.float32)
    u = np.asarray(u).astype(np.int64)
    v = np.asarray(v).astype(np.int64)
    widx = np.asarray(widx).astype(np.int64)

    (chunks, slot_base, col_base, TOT, NCOL, gidx_all, segid_all,
     cbase, cwidth) = _prep(u, v, widx)
    x16 = np.ascontiguousarray(x.astype(np.float16))
    wt_np = np.ascontiguousarray(
        np.transpose(W, (2, 0, 1)).astype(np.float16))  # [j, w, i]

    nc = _build_nc(chunks, slot_base, col_base, TOT, NCOL, cbase, cwidth)
    in_maps = [
        {"x16": x16, "wt": wt_np, "gidx": gidx_all[cc], "segid": segid_all[cc]}
        for cc in range(C)
    ]

    trace = bool(os.environ.get("KERNEL_TRACE"))
    LAST_RESULTS = run_bass_kernel_spmd(
        nc, in_maps, core_ids=list(range(C)),
        trace=trace, trace_cores=[0] if trace else None,
    )
    out = np.concatenate(
        [LAST_RESULTS.results[cc]["out"][:NPC] for cc in range(C)], axis=0)
    return out.astype(np.float32)


# revision 10
# speedup vs baseline: 2.4957x; 2.4957x over previous
"""GNN message-passing kernel for Trainium2 (8 NeuronCores).

out[v] = tanh( sum_w W[w] @ sum_{edges e: v_e=v, widx_e=w} x[u_e] )

Strategy (dest-sharded, gather-only, matmul segment-sum):
  - Nodes (destinations) sharded across 8 cores: core c owns v in
    [c*12500, (c+1)*12500).  x cast to fp16 host-side.
  - Edges bucketed host-side by (core, w, half, u_window, quad) where
    quad = 512 consecutive local dests; each cell padded to a multiple
    of 128 slots (cap shared across cores = SPMD).
  - Device, per (w, half): 4 dma_gather ops (one per 25k-row u-window)
    pull x[u_e] rows into SBUF staging in slot order.  Per quad q:
    chunks of 128 gathered rows are segment-summed via the Tensor
    engine:  S^T[j, seg] += G_chunk^T(stationary) @ onehot(segid)
    where onehot is built on DVE by comparing an iota row against the
    per-slot segment id (pad slots get segid=512 -> zero column).
    S^T accumulates in a PSUM bank [128, 512] across the quad's
    chunks; ACT copies it to SBUF (fp16); 4 weight matmuls
    (lhsT = S^T slice [j, v], rhs = W^T[j, i]) produce out[v, i]
    partials, accumulated over w into an SBUF f32 accumulator.
  - Final: tanh on ACT, DMA out.  No scatter, no S tables in HBM.
"""
import os
import numpy as np
import ml_dtypes

import concourse.bass as bass
import concourse.bacc as bacc
import concourse.mybir as mybir
import concourse.tile as tile
from concourse.bass_utils import run_bass_kernel_spmd

# problem shape (hardcoded per contract)
N, D, E, NW = 100000, 128, 2000000, 8
C = 8                  # cores
NPC = N // C           # 12500 dest nodes per core
WIN = 25000            # u gather window (int16-addressable rows)
NWIN = 4               # windows covering N
QS = 512               # dest quad size (one PSUM bank of segments)
NQ = 25                # quads per core (25*512 = 12800 >= 12500)
HALVES = [range(0, 13), range(13, 25)]  # quad halves per w
PAD_SEG = 512.0        # segid for pad slots (outside iota 0..511)

LAST_RESULTS = None    # BassKernelResults of the most recent run


def _wrap16(flat):
    """[n] -> [128, n/16] idx layout: position i at [i%16, i//16], replicated 8x."""
    base = flat.reshape(-1, 16).T  # [16, n/16]
    return np.tile(base, (8, 1))


def _prep(u, v, widx):
    """Bucket edges per core by (w, half, uwin, quad); shared caps across cores.

    Returns (caps[NW,NWIN,NQ] in chunks, gidx_all, segid_all, TOT, NCOL).
    """
    core = v // NPC
    vloc = v - core * NPC
    q = vloc // QS
    uw = u // WIN
    w = widx

    # counts per (core, w, uw, q) -> shared caps (in 128-chunks)
    lin = ((core * NW + w) * NWIN + uw) * NQ + q
    cnt = np.bincount(lin, minlength=C * NW * NWIN * NQ).reshape(C, NW, NWIN, NQ)
    chunks = -(-cnt.max(axis=0) // 128)          # [NW, NWIN, NQ] ceil
    chunks = np.maximum(chunks, 0)
    caps = chunks * 128

    # slot order: (w, half, uw, q-within-half); seg-col order: (w, q, uw)
    slot_cells = []   # (w, uw, q) in slot order
    for ww in range(NW):
        for half in HALVES:
            for uwin in range(NWIN):
                for qq in half:
                    slot_cells.append((ww, uwin, qq))
    col_cells = []
    for ww in range(NW):
        for qq in range(NQ):
            for uwin in range(NWIN):
                col_cells.append((ww, uwin, qq))

    slot_base = {}
    off = 0
    for cell in slot_cells:
        slot_base[cell] = off
        off += int(caps[cell[0], cell[1], cell[2]])
    TOT = off
    col_base = {}
    coff = 0
    for cell in col_cells:
        col_base[cell] = coff
        coff += int(chunks[cell[0], cell[1], cell[2]])
    NCOL = coff

    # per-chunk span union across cores -> (base, width) per segid column
    # col order: (w, q, uw, k);  smin/smax per (core, col)
    ncol_arr = np.zeros((C, coff, 2), np.int32)
    ncol_arr[:, :, 0] = QS    # smin init
    ncol_arr[:, :, 1] = -1    # smax init
    gidx_all, segid_all = [], []
    for cc in range(C):
        sel = core == cc
        ue = u[sel]
        vl = vloc[sel]
        we = w[sel]
        uwe = uw[sel]
        qe = q[sel]
        key = ((we * NWIN + uwe) * NQ + qe)
        order = np.lexsort((vl, key))
        ue, vl, we, uwe, qe, key = (a[order] for a in (ue, vl, we, uwe, qe, key))
        g_flat = np.zeros(TOT, np.int16)
        s_flat = np.full(NCOL * 128, PAD_SEG, np.float32)
        # per-cell fill via searchsorted on sorted keys
        bounds = np.searchsorted(key, np.arange(NW * NWIN * NQ + 1))
        for ww in range(NW):
            for uwin in range(NWIN):
                for qq in range(NQ):
                    b = (ww * NWIN + uwin) * NQ + qq
                    lo, hi = bounds[b], bounds[b + 1]
                    n = hi - lo
                    if n == 0:
                        continue
                    sb = slot_base[(ww, uwin, qq)]
                    cb = col_base[(ww, uwin, qq)]
                    assert n <= 128 * chunks[ww, uwin, qq]
                    g_flat[sb:sb + n] = (ue[lo:hi] - uwin * WIN).astype(np.int16)
                    s_flat[cb * 128:cb * 128 + n] = (vl[lo:hi] - qq * QS).astype(np.float32)
        gidx_all.append(_wrap16(g_flat))
        segid_all.append(s_flat.reshape(NCOL, 128))
        valid = s_flat.reshape(NCOL, 128) < QS
        sm = np.where(valid, s_flat.reshape(NCOL, 128), QS).min(axis=1)
        sx = np.where(valid, s_flat.reshape(NCOL, 128), -1).max(axis=1)
        ncol_arr[cc, :, 0] = sm
        ncol_arr[cc, :, 1] = sx
    smin = ncol_arr[:, :, 0].min(axis=0)
    smax = ncol_arr[:, :, 1].max(axis=0)
    smax = np.maximum(smax, 0)
    smin = np.minimum(smin, smax)
    base = np.minimum(smin // 128 * 128, QS - 128)
    width = (-(-(smax + 1 - base) // 128) * 128).clip(128, QS)
    base = np.minimum(base, QS - width)   # keep base+width <= QS
    # first chunk of every cell stays full-width (initializes its PSUM bank)
    first_cols = np.array(sorted(col_base.values()), int)
    first_cols = first_cols[first_cols < coff]
    base[first_cols] = 0
    width[first_cols] = QS
    # rebase segid planes; pad slots -> QS (never matches iota 0..width-1)
    segid_fin = []
    for cc in range(C):
        sp = segid_all[cc]            # [NCOL, 128]
        reb = np.where(sp < QS, sp - base[:, None], QS).astype(np.float32)
        segid_fin.append(np.ascontiguousarray(reb.T))
    segid_all = segid_fin
    return (chunks, slot_base, col_base, TOT, NCOL, gidx_all, segid_all,
            base.astype(int), width.astype(int))


def _build_nc(chunks, slot_base, col_base, TOT, NCOL, cbase, cwidth):
    nc = bacc.Bacc("TRN2", target_bir_lowering=False, debug=False, num_devices=C,
                   num_swdge_queues=4)
    fp16 = mybir.dt.float16
    f32 = mybir.dt.float32
    fp8 = mybir.dt.float8e4
    x_d = nc.dram_tensor("x16", [N, D], fp16, kind="ExternalInput")
    wt_d = nc.dram_tensor("wt", [D, NW, D], fp16, kind="ExternalInput")
    gidx_d = nc.dram_tensor("gidx", [128, TOT // 16], mybir.dt.int16,
                            kind="ExternalInput")
    segid_d = nc.dram_tensor("segid", [128, NCOL], f32, kind="ExternalInput")
    out_d = nc.dram_tensor("out", [NQ * QS, D], f32, kind="ExternalOutput")

    # per-op (w, half, uw): slot count and per-cell staging column offsets
    op_slots = {}
    stg_col = {}
    for ww in range(NW):
        for hh, half in enumerate(HALVES):
            for uwin in range(NWIN):
                s = 0
                for qq in half:
                    stg_col[(ww, uwin, qq)] = s
                    s += int(chunks[ww, uwin, qq])
                op_slots[(ww, hh, uwin)] = s * 128
    CMAX = max(op_slots.values()) // 128
    MAXCH = 0
    for ww in range(NW):
        for qq in range(NQ):
            MAXCH = max(MAXCH, int(chunks[ww, :, qq].sum()))

    with tile.TileContext(nc) as tc:
        with (
            tc.tile_pool(name="const", bufs=1) as constp,
            tc.tile_pool(name="outsb", bufs=1) as outp,
            tc.tile_pool(name="gip", bufs=8) as gip,
            tc.tile_pool(name="stgp", bufs=8) as stgp,
            tc.tile_pool(name="indp", bufs=12) as indp,
            tc.tile_pool(name="stp", bufs=4) as stp,
            tc.tile_pool(name="psA", bufs=3, space="PSUM") as psA,
            tc.tile_pool(name="psB", bufs=3, space="PSUM") as psB,
        ):
            iota_t = constp.tile([128, QS], fp16)
            nc.gpsimd.iota(iota_t[:], pattern=[[1, QS]], base=0,
                           channel_multiplier=0,
                           allow_small_or_imprecise_dtypes=True)
            wt_t = constp.tile([128, NW, D], fp16)
            nc.sync.dma_start(out=wt_t[:], in_=wt_d[:])
            segid_t = constp.tile([128, NCOL], f32)
            nc.sync.dma_start(out=segid_t[:], in_=segid_d[:])
            out_sb = []
            for qq in range(NQ):
                t = outp.tile([128, QS], f32, tag=f"out{qq}")
                nc.vector.memset(t[:], 0.0)
                out_sb.append(t)

            gcol = [0]  # running segid column, (w, q, uw, k) order

            slot_off = 0
            for ww in range(NW):
                for hh, half in enumerate(HALVES):
                    stg = {}
                    for uwin in range(NWIN):
                        nslots = op_slots[(ww, hh, uwin)]
                        if nslots == 0:
                            continue
                        gi = gip.tile([128, CMAX * 8], mybir.dt.int16, tag="gi")
                        nc.sync.dma_start(
                            out=gi[:, :nslots // 16],
                            in_=gidx_d[:, slot_off // 16:(slot_off + nslots) // 16])
                        st = stgp.tile([128, CMAX, D], fp16, tag="stg")
                        nc.gpsimd.dma_gather(
                            st[:, :nslots // 128, :],
                            x_d[uwin * WIN:min((uwin + 1) * WIN, N)],
                            gi[:, :nslots // 16], nslots, nslots, D,
                            single_packet=False, queue_num=uwin)
                        stg[uwin] = st
                        slot_off += nslots
                    for qq in half:
                        cell_list = []
                        for uwin in range(NWIN):
                            nch = int(chunks[ww, uwin, qq])
                            for k in range(nch):
                                cell_list.append((uwin, stg_col[(ww, uwin, qq)] + k))
                        if not cell_list:
                            continue
                        acc = psA.tile([128, QS], f32, tag="acc")
                        for i, (uwin, colk) in enumerate(cell_list):
                            g = gcol[0]
                            gcol[0] += 1
                            b, wdt = int(cbase[g]), int(cwidth[g])
                            if i == 0:
                                assert (b, wdt) == (0, QS)
                            ind = indp.tile([128, QS], fp16, tag="ind")
                            nc.vector.tensor_scalar(
                                ind[:, :wdt], iota_t[:, :wdt],
                                segid_t[:, g:g + 1], None,
                                mybir.AluOpType.is_equal)
                            nc.tensor.matmul(
                                out=acc[:, b:b + wdt],
                                lhsT=stg[uwin][:, colk, :],
                                rhs=ind[:, :wdt],
                                start=(i == 0), stop=(i == len(cell_list) - 1),
                                skip_group_check=True)
                        stq = stp.tile([128, QS], fp16, tag="st")
                        nc.scalar.activation(stq[:], acc[:],
                                             mybir.ActivationFunctionType.Copy)
                        wp = psB.tile([128, QS], f32, tag="wp")
                        for vt in range(4):
                            nc.tensor.matmul(
                                out=wp[:, vt * 128:(vt + 1) * 128],
                                lhsT=stq[:, vt * 128:(vt + 1) * 128],
                                rhs=wt_t[:, ww, :], start=True, stop=True)
                        nc.vector.tensor_tensor(
                            out=out_sb[qq][:], in0=out_sb[qq][:], in1=wp[:],
                            op=mybir.AluOpType.add)

            # final tanh + store
            with tc.tile_pool(name="fin", bufs=4) as finp:
                for qq in range(NQ):
                    for vt in range(4):
                        ot = finp.tile([128, 128], f32, tag="ot")
                        nc.scalar.activation(
                            ot[:], out_sb[qq][:, vt * 128:(vt + 1) * 128],
                            mybir.ActivationFunctionType.Tanh)
                        r = qq * QS + vt * 128
                        nc.sync.dma_start(out=out_d[r:r + 128, :], in_=ot[:])

    nc.compile()
    return nc


def kernel(x, W, u, v, widx):
    global LAST_RESULTS
    x = np.asarray(x, dtype=np.float32)
    W = np.asarray(W, dtype=np.float32)
    u = np.asarray(u).astype(np.int64)
    v = np.asarray(v).astype(np.int64)
    widx = np.asarray(widx).astype(np.int64)

    (chunks, slot_base, col_base, TOT, NCOL, gidx_all, segid_all,
     cbase, cwidth) = _prep(u, v, widx)
    x16 = np.ascontiguousarray(x.astype(np.float16))
    wt_np = np.ascontiguousarray(
        np.transpose(W, (2, 0, 1)).astype(np.float16))  # [j, w, i]

    nc = _build_nc(chunks, slot_base, col_base, TOT, NCOL, cbase, cwidth)
    in_maps = [
        {"x16": x16, "wt": wt_np, "gidx": gidx_all[cc], "segid": segid_all[cc]}
        for cc in range(C)
    ]

    trace = bool(os.environ.get("KERNEL_TRACE"))
    LAST_RESULTS = run_bass_kernel_spmd(
        nc, in_maps, core_ids=list(range(C)),
        trace=trace, trace_cores=[0] if trace else None,
    )
    out = np.concatenate(
        [LAST_RESULTS.results[cc]["out"][:NPC] for cc in range(C)], axis=0)
    return out.astype(np.float32)
